# revision 24
# baseline (speedup 1.0000x reference)
"""Trainium2 Bass kernel: int8-LUT-emulated 3x3 Conv2d (B=4, Cin=Cout=64, 28x28).

The LUT passed by the problem generator is the exact int8 product table
lut[i, j] = (i-128)*(j-128), so the gather-accumulate in the reference is
mathematically an integer matmul of the quantized activations and weights.
Quantized values lie in [-128, 127]; they are exactly representable in bf16,
bf16 products are exact in fp32, and the accumulated sums stay below 2^24 -
so a bf16 tensor-engine matmul with fp32 PSUM accumulation reproduces the
reference bit-exactly (up to the reciprocal-vs-divide ulp in the scale).

Sharding (8 cores): data-parallel over batch (4) x spatial halves (2).
Each core computes out[b, :, h*14:(h+1)*14, :] = [64, 14, 28].

v6 pipeline, built around the ~300 GB/s aggregate DMA wall:
  - inputs spread over three DGE queues (sync, scalar, gpsimd); the
    gpsimd custom-op library load measures ~9.5us, so NO gpsimd custom
    ops anywhere - cross-partition maxima go through PE transposes and a
    broadcast matmul against a shipped identity (ready by ~10us).
  - w scale chain runs early (wtx lands first): wq clears the ACT engine
    long before the x scale exists.
  - bias rides in an unused wtx slot scaled by 2^-40 (exact), unscaled by
    one DVE op; no separate bias DMA (64 x 4B packets cost engine slots).
  - x quantize on DVE, dequant split ACT || DVE, output on both rings.
"""

import numpy as np

import concourse.bacc as bacc
import concourse.mybir as mybir
import concourse.tile as tile
from concourse.bass_utils import run_bass_kernel_spmd

F32 = mybir.dt.float32
BF16 = mybir.dt.bfloat16
ALU = mybir.AluOpType
AX = mybir.AxisListType
ACT_ID = mybir.ActivationFunctionType.Identity

B, C, H, W = 4, 64, 28, 28
COUT, KS, PAD = 64, 3, 1
QMAX = 127.0
MAGIC = 12582912.0  # 1.5 * 2**23: fp32 add/sub rounds to nearest-even integer
BSCALE = 2.0 ** 40  # bias hides in wtx scaled by 2^-40 (exact, absmax-invisible)

HALF = 14          # output rows per core
XB_ROWS = 16       # padded input rows held per half (14 outputs need 16 rows)
PW = W + 2 * PAD   # 30
XR_COLS = 1358     # ceil(leftover-x elements / 128); zero-padded
N_CORES = 8

# xr column chunks: sync, scalar, gpsimd queues (one large-elem descriptor
# each - small per-partition elem sizes measurably tank DGE throughput)
XA, XB = 500, 280    # sync / scalar; gpsimd takes the rest (578 cols)
CA = XA
CB = XA + XB


def _build_bass():
    nc = bacc.Bacc(None)

    xb2s_d = nc.dram_tensor("xb2s", [64, XB_ROWS + 1, PW], F32, kind="ExternalInput")
    xr_d = nc.dram_tensor("xr", [128, XR_COLS], F32, kind="ExternalInput")
    wtx_d = nc.dram_tensor("wtx", [128, 6, COUT], F32, kind="ExternalInput")
    idt_d = nc.dram_tensor("idt", [128, 128], F32, kind="ExternalInput")
    out_d = nc.dram_tensor("out", [COUT, HALF, W], F32, kind="ExternalOutput")

    with tile.TileContext(nc) as tc:
        with (
            tc.tile_pool(name="p", bufs=1) as pool,
            tc.tile_pool(name="ps", bufs=1, space="PSUM") as psum,
        ):
            xb2s = pool.tile([64, XB_ROWS + 1, PW], F32, tag="xb2s")
            xb2 = pool.tile([128, XB_ROWS, PW], F32, tag="xb2")
            xr = pool.tile([128, XR_COLS], F32, tag="xr")
            wtx = pool.tile([128, 6, COUT], F32, tag="wtx")
            mx = pool.tile([128, 5], F32, tag="mx")        # x absmax partials
            mw = pool.tile([128, 1], F32, tag="mw")        # w absmax partials
            mxc = pool.tile([128, 1], F32, tag="mxc")      # combined x partials
            idt = pool.tile([128, 128], F32, tag="idt")
            ones1 = pool.tile([1, 128], F32, tag="ones1")
            # p0 scalar slots: 0=sw 1=1/sw 2=x_raw 3=sx 4=1/sx 5=sx*sw
            sxv = pool.tile([1, 8], F32, tag="sxv")
            scw = pool.tile([128, 1], F32, tag="scw")      # bcast 1/sw
            scx = pool.tile([128, 2], F32, tag="scx")      # bcast [1/sx, sx*sw]
            biasur = pool.tile([COUT, 1], F32, tag="biasur")
            tw = pool.tile([128, 6, COUT], F32, tag="tw")
            wq = pool.tile([128, 6, COUT], BF16, tag="wq")
            tx = pool.tile([128, XB_ROWS, PW], F32, tag="tx")
            xq = pool.tile([128, XB_ROWS, PW], BF16, tag="xq")
            outs = pool.tile([COUT, HALF, W], F32, tag="outs")
            magict = pool.tile([128, 1], F32, tag="magict")
            nmagict = pool.tile([128, 1], F32, tag="nmagict")

            cps = psum.tile([COUT, HALF, W], F32, tag="cps")
            tpw = psum.tile([1, 128], F32, tag="tpw")
            tpx = psum.tile([1, 128], F32, tag="tpx")
            bcw = psum.tile([128, 1], F32, tag="bcw")
            bcx = psum.tile([128, 2], F32, tag="bcx")

            # --- DMA issues; most-critical tensor leads each queue.  The
            # identity (needed by the PE transposes ~12us) leads gpsimd's.
            nc.scalar.dma_start(wtx[:], wtx_d[:])
            nc.sync.dma_start(xb2s[:], xb2s_d[:])
            nc.gpsimd.dma_start(idt[:], idt_d[:])
            nc.gpsimd.dma_start(xr[:, CB:XR_COLS], xr_d[:, CB:XR_COLS])
            nc.scalar.dma_start(xr[:, CA:CB], xr_d[:, CA:CB])
            nc.sync.dma_start(xr[:, 0:CA], xr_d[:, 0:CA])
            # build the kh-merged 128-partition layout from the slab with
            # local SBUF->SBUF DMAs (no extra HBM reads; they queue behind
            # the HBM pulls and finish well before quantize needs them)
            nc.sync.dma_start(xb2[0:64, :, :], xb2s[:, 0:XB_ROWS, :])
            nc.sync.dma_start(xb2[64:128, :, :], xb2s[:, 1:XB_ROWS + 1, :])

            nc.gpsimd.memset(mx[:], 0.0)
            nc.gpsimd.memset(magict[:], MAGIC)
            nc.gpsimd.memset(nmagict[:], -MAGIC)
            nc.gpsimd.memset(ones1[:], 1.0)
            # Preload the scalar engine's activation table off the critical
            # path (first ACT use otherwise pays ~1.3us mid-kernel).
            nc.scalar.activation(magict[0:1, 0:1], magict[0:1, 0:1], ACT_ID)

            # --- bias unscale (wtx slot [0:64, 3, 0] holds bias * 2^-40)
            nc.vector.tensor_scalar(
                biasur[:], wtx[0:COUT, 3, 0:1], BSCALE, None, op0=ALU.mult)

            # --- absmax partials (DVE, scheduler runs them by DMA arrival)
            nc.vector.tensor_reduce(
                mw[:], wtx[:], axis=AX.XY, op=ALU.max,
                apply_absolute_value=True)
            # --- w scale chain EARLY (wtx + identity land first): PE
            # transpose -> p0 scalar math -> broadcast matmul; wq then fully
            # clears the ACT engine before the x scale exists.
            nc.tensor.transpose(tpw[:], mw[:], idt[:])
            nc.vector.tensor_reduce(
                sxv[0:1, 0:1], tpw[:], axis=AX.X, op=ALU.max)
            nc.vector.tensor_scalar(
                sxv[0:1, 1:2], sxv[0:1, 0:1], 1.0 / QMAX, None, op0=ALU.mult)
            nc.vector.reciprocal(sxv[0:1, 2:3], sxv[0:1, 1:2])
            nc.tensor.matmul(
                bcw[:], ones1[:], sxv[0:1, 2:3], start=True, stop=True)
            nc.scalar.activation(
                scw[:], bcw[:], mybir.ActivationFunctionType.Copy)
            nc.scalar.activation(
                tw[:, 0:3, :], wtx[:, 0:3, :], ACT_ID,
                bias=magict[:], scale=scw[:])
            nc.scalar.activation(
                wq[:, 0:3, :], tw[:, 0:3, :], ACT_ID, bias=nmagict[:])
            nc.scalar.activation(
                tw[:, 3:6, :], wtx[:, 3:6, :], ACT_ID,
                bias=magict[:], scale=scw[:])
            nc.scalar.activation(
                wq[:, 3:6, :], tw[:, 3:6, :], ACT_ID, bias=nmagict[:])

            nc.vector.tensor_reduce(
                mx[0:64, 0:1], xb2s[:], axis=AX.XY, op=ALU.max,
                apply_absolute_value=True)
            nc.vector.tensor_reduce(
                mx[:, 1:2], xr[:, 0:CA], axis=AX.X, op=ALU.max,
                apply_absolute_value=True)
            nc.vector.tensor_reduce(
                mx[:, 2:3], xr[:, CA:CB], axis=AX.X, op=ALU.max,
                apply_absolute_value=True)
            nc.vector.tensor_reduce(
                mx[:, 3:4], xr[:, CB:XR_COLS], axis=AX.X, op=ALU.max,
                apply_absolute_value=True)
            nc.vector.tensor_reduce(
                mxc[:], mx[:, 0:4], axis=AX.X, op=ALU.max)

            # --- x scale chain: transpose, p0 scalars, ONE broadcast matmul
            # carrying [1/sx, sx*sw] in its two moving columns
            nc.tensor.transpose(tpx[:], mxc[:], idt[:])
            nc.vector.tensor_reduce(
                sxv[0:1, 3:4], tpx[:], axis=AX.X, op=ALU.max)
            nc.vector.tensor_scalar(
                sxv[0:1, 4:5], sxv[0:1, 3:4], 1.0 / QMAX, None, op0=ALU.mult)
            nc.vector.reciprocal(sxv[0:1, 5:6], sxv[0:1, 4:5])
            nc.vector.tensor_tensor(
                sxv[0:1, 6:7], sxv[0:1, 4:5], sxv[0:1, 1:2], op=ALU.mult)
            nc.tensor.matmul(
                bcx[:], ones1[:], sxv[0:1, 5:7], start=True, stop=True)
            nc.vector.tensor_copy(scx[:], bcx[:])

            # --- x quantize on DVE (ACT per-op overhead dwarfs its help)
            nc.vector.tensor_scalar(
                tx[:], xb2[:], scx[:, 0:1], MAGIC, op0=ALU.mult, op1=ALU.add)
            nc.vector.tensor_scalar(
                xq[:], tx[:], MAGIC, None, op0=ALU.subtract)

            # --- conv: 6 accumulating matmuls
            # partitions 0..63 hold padded rows r0..r0+15 (kh=0), partitions
            # 64..127 hold rows r0+1..r0+16 (kh=1 at the same row slice; kh=2
            # one slice down).
            for kw in range(3):
                nc.tensor.matmul(
                    cps[:], wq[:, kw, :], xq[:, 0:HALF, kw:kw + W],
                    start=(kw == 0), stop=False)
            for kw in range(3):
                nc.tensor.matmul(
                    cps[:], wq[64:128, 3 + kw, :], xq[64:128, 1:HALF + 1, kw:kw + W],
                    start=False, stop=(kw == 2))

            # --- dequantize + bias: ACT || DVE, output halves on both rings
            HH = HALF // 2
            nc.vector.tensor_scalar(
                outs[:], cps[:],
                scx[0:COUT, 1:2], biasur[:], op0=ALU.mult, op1=ALU.add)
            nc.sync.dma_start(out_d[:, 0:HH, :], outs[:, 0:HH, :])
            nc.scalar.dma_start(out_d[:, HH:HALF, :], outs[:, HH:HALF, :])

    nc.compile()
    return nc


_NC_CACHE = None


def _get_nc():
    global _NC_CACHE
    if _NC_CACHE is None:
        _NC_CACHE = _build_bass()
    return _NC_CACHE


def make_in_maps(x, weight, bias):
    x = np.ascontiguousarray(x, np.float32)
    weight = np.ascontiguousarray(weight, np.float32)

    # padded x with two extra zero rows so the row-shifted copy can slice
    xpad = np.zeros((B, C, H + 4, PW), np.float32)
    xpad[:, :, 1:1 + H, 1:1 + W] = x

    wt = weight.transpose(1, 2, 3, 0)  # [cin, kh, kw, cout]
    wtx = np.zeros((128, 6, COUT), np.float32)
    wtx[:64, 0:3] = wt[:, 0]
    wtx[64:, 0:3] = wt[:, 1]
    wtx[64:, 3:6] = wt[:, 2]
    # bias hides in the unused (0:64, tap 3) slot, scaled to be invisible
    # to the w absmax (2^-40 is exact in fp32)
    wtx[0:COUT, 3, 0] = bias.astype(np.float32) / BSCALE
    idt = np.eye(128, dtype=np.float32)

    in_maps = []
    for core in range(N_CORES):
        b, h = divmod(core, 2)
        r0 = h * HALF
        xb2s = np.ascontiguousarray(xpad[b, :, r0:r0 + XB_ROWS + 1, :])

        # rows of batch b not covered by xb2, plus the other three batches,
        # packed for the replicated global absmax
        left_rows = range(XB_ROWS, H) if h == 0 else range(0, HALF - 1)
        leftover = x[b][:, list(left_rows), :].ravel()
        others = np.delete(x, b, axis=0).ravel()
        xr = np.zeros(128 * XR_COLS, np.float32)
        fill = np.concatenate([leftover, others])
        xr[:fill.size] = fill

        in_maps.append({
            "xb2s": xb2s,
            "xr": xr.reshape(128, XR_COLS),
            "wtx": wtx,
            "idt": idt,
        })
    return in_maps


def assemble_output(results):
    out = np.empty((B, COUT, H, W), np.float32)
    for core in range(N_CORES):
        b, h = divmod(core, 2)
        out[b, :, h * HALF:(h + 1) * HALF, :] = results[core]["out"]
    return out


def kernel(x, weight, bias, lut, **run_kwargs):
    nc = _get_nc()
    in_maps = make_in_maps(x, weight, bias)
    res = run_bass_kernel_spmd(nc, in_maps, list(range(N_CORES)), **run_kwargs)
    out = assemble_output(res.results)
    kernel.last_result = res
    return out


# revision 25
# speedup vs baseline: 1.0292x; 1.0292x over previous
"""Trainium2 Bass kernel: int8-LUT-emulated 3x3 Conv2d (B=4, Cin=Cout=64, 28x28).

The LUT passed by the problem generator is the exact int8 product table
lut[i, j] = (i-128)*(j-128), so the gather-accumulate in the reference is
mathematically an integer matmul of the quantized activations and weights.
Quantized values lie in [-128, 127]; they are exactly representable in bf16,
bf16 products are exact in fp32, and the accumulated sums stay below 2^24 -
so a bf16 tensor-engine matmul with fp32 PSUM accumulation reproduces the
reference bit-exactly (up to the reciprocal-vs-divide ulp in the scale).

Sharding (8 cores): data-parallel over batch (4) x spatial halves (2).
Each core computes out[b, :, h*14:(h+1)*14, :] = [64, 14, 28].

v6 pipeline, built around the ~300 GB/s aggregate DMA wall:
  - inputs spread over three DGE queues (sync, scalar, gpsimd); the
    gpsimd custom-op library load measures ~9.5us, so NO gpsimd custom
    ops anywhere - cross-partition maxima go through PE transposes and a
    broadcast matmul against a shipped identity (ready by ~10us).
  - w scale chain runs early (wtx lands first): wq clears the ACT engine
    long before the x scale exists.
  - bias rides in an unused wtx slot scaled by 2^-40 (exact), unscaled by
    one DVE op; no separate bias DMA (64 x 4B packets cost engine slots).
  - x quantize on DVE, dequant split ACT || DVE, output on both rings.
"""

import numpy as np

import concourse.bacc as bacc
import concourse.mybir as mybir
import concourse.tile as tile
from concourse.bass_utils import run_bass_kernel_spmd

F32 = mybir.dt.float32
BF16 = mybir.dt.bfloat16
ALU = mybir.AluOpType
AX = mybir.AxisListType
ACT_ID = mybir.ActivationFunctionType.Identity

B, C, H, W = 4, 64, 28, 28
COUT, KS, PAD = 64, 3, 1
QMAX = 127.0
MAGIC = 12582912.0  # 1.5 * 2**23: fp32 add/sub rounds to nearest-even integer
BSCALE = 2.0 ** 40  # bias hides in wtx scaled by 2^-40 (exact, absmax-invisible)

HALF = 14          # output rows per core
XB_ROWS = 16       # padded input rows held per half (14 outputs need 16 rows)
PW = W + 2 * PAD   # 30
XR_COLS = 1358     # ceil(leftover-x elements / 128); zero-padded
N_CORES = 8

# xr column chunks: sync, scalar, gpsimd queues (one large-elem descriptor
# each - small per-partition elem sizes measurably tank DGE throughput)
XA, XB = 560, 220    # sync / scalar; gpsimd takes the rest (578 cols)
CA = XA
CB = XA + XB


def _build_bass():
    nc = bacc.Bacc(None)

    xb2_d = nc.dram_tensor("xb2", [128, XB_ROWS, PW], F32, kind="ExternalInput")
    xr_d = nc.dram_tensor("xr", [128, XR_COLS], F32, kind="ExternalInput")
    wtx_d = nc.dram_tensor("wtx", [128, 6, COUT], F32, kind="ExternalInput")
    idt_d = nc.dram_tensor("idt", [128, 128], F32, kind="ExternalInput")
    out_d = nc.dram_tensor("out", [COUT, HALF, W], F32, kind="ExternalOutput")

    with tile.TileContext(nc) as tc:
        with (
            tc.tile_pool(name="p", bufs=1) as pool,
            tc.tile_pool(name="ps", bufs=1, space="PSUM") as psum,
        ):
            xb2 = pool.tile([128, XB_ROWS, PW], F32, tag="xb2")
            xr = pool.tile([128, XR_COLS], F32, tag="xr")
            wtx = pool.tile([128, 6, COUT], F32, tag="wtx")
            mx = pool.tile([128, 5], F32, tag="mx")        # x absmax partials
            mw = pool.tile([128, 1], F32, tag="mw")        # w absmax partials
            mxc = pool.tile([128, 1], F32, tag="mxc")      # combined x partials
            idt = pool.tile([128, 128], F32, tag="idt")
            ones1 = pool.tile([1, 128], F32, tag="ones1")
            # p0 scalar slots: 0=sw 1=1/sw 2=x_raw 3=sx 4=1/sx 5=sx*sw
            sxv = pool.tile([1, 8], F32, tag="sxv")
            scw = pool.tile([128, 1], F32, tag="scw")      # bcast 1/sw
            biasur = pool.tile([COUT, 1], F32, tag="biasur")
            tw = pool.tile([128, 6, COUT], F32, tag="tw")
            wq = pool.tile([128, 6, COUT], BF16, tag="wq")
            tx = pool.tile([128, XB_ROWS, PW], F32, tag="tx")
            xq = pool.tile([128, XB_ROWS, PW], BF16, tag="xq")
            outs = pool.tile([COUT, HALF, W], F32, tag="outs")
            magict = pool.tile([128, 1], F32, tag="magict")
            nmagict = pool.tile([128, 1], F32, tag="nmagict")

            cps = psum.tile([COUT, HALF, W], F32, tag="cps")
            tpw = psum.tile([1, 128], F32, tag="tpw")
            tpx = psum.tile([1, 128], F32, tag="tpx")
            bcw = psum.tile([128, 1], F32, tag="bcw")
            bcx = psum.tile([128, 2], F32, tag="bcx")

            # --- DMA issues; most-critical tensor leads each queue.  The
            # identity (needed by the PE transposes ~12us) leads gpsimd's.
            nc.scalar.dma_start(wtx[:], wtx_d[:])
            nc.sync.dma_start(xb2[:], xb2_d[:])
            nc.gpsimd.dma_start(idt[:], idt_d[:])
            nc.gpsimd.dma_start(xr[:, CB:XR_COLS], xr_d[:, CB:XR_COLS])
            nc.scalar.dma_start(xr[:, CA:CB], xr_d[:, CA:CB])
            nc.sync.dma_start(xr[:, 0:CA], xr_d[:, 0:CA])

            nc.gpsimd.memset(magict[:], MAGIC)
            nc.gpsimd.memset(nmagict[:], -MAGIC)
            nc.gpsimd.memset(ones1[:], 1.0)
            # Preload the scalar engine's activation table off the critical
            # path (first ACT use otherwise pays ~1.3us mid-kernel).
            nc.scalar.activation(magict[0:1, 0:1], magict[0:1, 0:1], ACT_ID)

            # --- bias unscale (wtx slot [0:64, 3, 0] holds bias * 2^-40)
            nc.vector.tensor_scalar(
                biasur[:], wtx[0:COUT, 3, 0:1], BSCALE, None, op0=ALU.mult)

            # --- absmax partials (DVE, scheduler runs them by DMA arrival)
            nc.vector.tensor_reduce(
                mw[:], wtx[:], axis=AX.XY, op=ALU.max,
                apply_absolute_value=True)
            # --- w scale chain EARLY (wtx + identity land first): PE
            # transpose -> p0 scalar math -> broadcast matmul; wq then fully
            # clears the ACT engine before the x scale exists.
            nc.tensor.transpose(tpw[:], mw[:], idt[:])
            nc.vector.tensor_reduce(
                sxv[0:1, 0:1], tpw[:], axis=AX.X, op=ALU.max)
            nc.vector.tensor_scalar(
                sxv[0:1, 1:2], sxv[0:1, 0:1], 1.0 / QMAX, None, op0=ALU.mult)
            nc.vector.reciprocal(sxv[0:1, 2:3], sxv[0:1, 1:2])
            nc.tensor.matmul(
                bcw[:], ones1[:], sxv[0:1, 2:3], start=True, stop=True)
            nc.scalar.activation(
                scw[:], bcw[:], mybir.ActivationFunctionType.Copy)
            nc.scalar.activation(
                tw[:, 0:3, :], wtx[:, 0:3, :], ACT_ID,
                bias=magict[:], scale=scw[:])
            nc.scalar.activation(
                wq[:, 0:3, :], tw[:, 0:3, :], ACT_ID, bias=nmagict[:])
            nc.scalar.activation(
                tw[:, 3:6, :], wtx[:, 3:6, :], ACT_ID,
                bias=magict[:], scale=scw[:])
            nc.scalar.activation(
                wq[:, 3:6, :], tw[:, 3:6, :], ACT_ID, bias=nmagict[:])

            nc.vector.tensor_reduce(
                mx[:, 0:1], xb2[:], axis=AX.XY, op=ALU.max,
                apply_absolute_value=True)
            nc.vector.tensor_reduce(
                mx[:, 1:2], xr[:, CB:XR_COLS], axis=AX.X, op=ALU.max,
                apply_absolute_value=True)
            nc.vector.tensor_reduce(
                mx[:, 2:3], xr[:, 0:CA], axis=AX.X, op=ALU.max,
                apply_absolute_value=True)
            nc.vector.tensor_reduce(
                mx[:, 3:4], xr[:, CA:CB], axis=AX.X, op=ALU.max,
                apply_absolute_value=True)
            nc.vector.tensor_reduce(
                mxc[:], mx[:, 0:4], axis=AX.X, op=ALU.max)

            # --- x scale chain: transpose, p0 scalars, ONE broadcast matmul
            # carrying [1/sx, sx*sw] in its two moving columns
            nc.tensor.transpose(tpx[:], mxc[:], idt[:])
            nc.vector.tensor_reduce(
                sxv[0:1, 3:4], tpx[:], axis=AX.X, op=ALU.max)
            nc.vector.tensor_scalar(
                sxv[0:1, 4:5], sxv[0:1, 3:4], 1.0 / QMAX, None, op0=ALU.mult)
            nc.vector.reciprocal(sxv[0:1, 5:6], sxv[0:1, 4:5])
            nc.vector.tensor_tensor(
                sxv[0:1, 6:7], sxv[0:1, 4:5], sxv[0:1, 1:2], op=ALU.mult)
            nc.tensor.matmul(
                bcx[:], ones1[:], sxv[0:1, 5:7], start=True, stop=True)

            # --- x quantize on DVE (ACT per-op overhead dwarfs its help)
            nc.vector.tensor_scalar(
                tx[:], xb2[:], bcx[:, 0:1], MAGIC, op0=ALU.mult, op1=ALU.add)
            nc.vector.tensor_scalar(
                xq[:], tx[:], MAGIC, None, op0=ALU.subtract)

            # --- conv: 6 accumulating matmuls
            # partitions 0..63 hold padded rows r0..r0+15 (kh=0), partitions
            # 64..127 hold rows r0+1..r0+16 (kh=1 at the same row slice; kh=2
            # one slice down).
            for kw in range(3):
                nc.tensor.matmul(
                    cps[:], wq[:, kw, :], xq[:, 0:HALF, kw:kw + W],
                    start=(kw == 0), stop=False)
            for kw in range(3):
                nc.tensor.matmul(
                    cps[:], wq[64:128, 3 + kw, :], xq[64:128, 1:HALF + 1, kw:kw + W],
                    start=False, stop=(kw == 2))

            # --- dequantize + bias: ACT || DVE, output halves on both rings
            HH = HALF // 2
            nc.vector.tensor_scalar(
                outs[:], cps[:],
                bcx[0:COUT, 1:2], biasur[:], op0=ALU.mult, op1=ALU.add)
            nc.sync.dma_start(out_d[:, 0:HH, :], outs[:, 0:HH, :])
            nc.scalar.dma_start(out_d[:, HH:HALF, :], outs[:, HH:HALF, :])

    nc.compile()
    return nc


_NC_CACHE = None


def _get_nc():
    global _NC_CACHE
    if _NC_CACHE is None:
        _NC_CACHE = _build_bass()
    return _NC_CACHE


def make_in_maps(x, weight, bias):
    x = np.ascontiguousarray(x, np.float32)
    weight = np.ascontiguousarray(weight, np.float32)

    # padded x with two extra zero rows so the row-shifted copy can slice
    xpad = np.zeros((B, C, H + 4, PW), np.float32)
    xpad[:, :, 1:1 + H, 1:1 + W] = x

    wt = weight.transpose(1, 2, 3, 0)  # [cin, kh, kw, cout]
    wtx = np.zeros((128, 6, COUT), np.float32)
    wtx[:64, 0:3] = wt[:, 0]
    wtx[64:, 0:3] = wt[:, 1]
    wtx[64:, 3:6] = wt[:, 2]
    # bias hides in the unused (0:64, tap 3) slot, scaled to be invisible
    # to the w absmax (2^-40 is exact in fp32)
    wtx[0:COUT, 3, 0] = bias.astype(np.float32) / BSCALE
    idt = np.eye(128, dtype=np.float32)

    in_maps = []
    for core in range(N_CORES):
        b, h = divmod(core, 2)
        r0 = h * HALF
        xb_lo = xpad[b, :, r0:r0 + XB_ROWS, :]
        xb_hi = xpad[b, :, r0 + 1:r0 + 1 + XB_ROWS, :]
        xb2 = np.ascontiguousarray(np.concatenate([xb_lo, xb_hi], axis=0))

        # rows of batch b not covered by xb2, plus the other three batches,
        # packed for the replicated global absmax
        left_rows = range(XB_ROWS, H) if h == 0 else range(0, HALF - 1)
        leftover = x[b][:, list(left_rows), :].ravel()
        others = np.delete(x, b, axis=0).ravel()
        xr = np.zeros(128 * XR_COLS, np.float32)
        fill = np.concatenate([leftover, others])
        xr[:fill.size] = fill

        in_maps.append({
            "xb2": xb2,
            "xr": xr.reshape(128, XR_COLS),
            "wtx": wtx,
            "idt": idt,
        })
    return in_maps


def assemble_output(results):
    out = np.empty((B, COUT, H, W), np.float32)
    for core in range(N_CORES):
        b, h = divmod(core, 2)
        out[b, :, h * HALF:(h + 1) * HALF, :] = results[core]["out"]
    return out


def kernel(x, weight, bias, lut, **run_kwargs):
    nc = _get_nc()
    in_maps = make_in_maps(x, weight, bias)
    res = run_bass_kernel_spmd(nc, in_maps, list(range(N_CORES)), **run_kwargs)
    out = assemble_output(res.results)
    kernel.last_result = res
    return out
